# revision 28
# baseline (speedup 1.0000x reference)
"""Trainium2 Bass kernel for the AFA-GNN message-passing network (8 NeuronCores).

Math (the tanh gate in the reference is dead code — overwritten with -1):
    deg = bincount(row);  nd = clip(deg,1)^-0.5;  g1_e = nd[row_e]*nd[col_e]
    raw  = relu(x @ W1.T + b1)
    sraw = (sigmoid(eps1)+1) * raw
    h1   = sraw - segsum_col(g1 * raw[row])
    h2   = sraw - segsum_col(g1 * h1[row])
    out  = (log_softmax(h2 @ W2.T + b2), h2)

Distribution: nodes (and their incoming edges, i.e. col-sharded) split over 8
cores.  Each core computes its node-shard of raw/sraw, pre-scales by -nd[row],
AllGathers the scaled table, then for each 128-col group gathers the table
rows of its edges (indirect DMA), builds a 0/1 col-selection matrix on the
vector engine, and segment-sums via matmul accumulation in PSUM.  The group
aggregate lands directly in SBUF node-major — no scatter is ever needed.
"""

import numpy as np

import concourse.bass as bass
import concourse.bacc as bacc
import concourse.tile as tile
from concourse import mybir, bass_utils
from concourse.masks import make_identity

F32 = mybir.dt.float32
BF16 = mybir.dt.bfloat16
I32 = mybir.dt.int32


class Cfg:
    def __init__(self, N, E, F, H, C, ncores=8):
        self.N, self.E, self.F, self.H, self.C = N, E, F, H, C
        self.ncores = ncores
        assert N % ncores == 0
        self.NPCR = N // ncores                 # real nodes per core
        self.NPC = ((self.NPCR + 127) // 128) * 128  # padded nodes per core
        # pad edges gather table row NPCR, which must be a zero pad row
        assert self.NPC > self.NPCR
        self.NB = self.NPC // 128               # node blocks == col groups
        self.FC = F // 128                      # feature chunks
        self.NFULL = ncores * self.NPC          # padded global table rows


REAL = Cfg(N=100000, E=1600000, F=512, H=64, C=40)


def _balance_cols(din, NG, cap_main):
    """Assign each local col to a group of exactly 128 cols.  Groups
    0..NG-2 are balanced (LPT) around cap_main edges; the core's excess
    beyond (NG-1)*cap_main is concentrated into group NG-1 so that the
    max-over-cores subtile count inflates a single group id only.
    Returns perm: old local col -> new local col."""
    npc = len(din)
    total = int(din.sum())

    # pick 128 cols for the remainder group with degree sum ~ T
    T = max(0, total - (NG - 1) * cap_main)
    order = np.argsort(din, kind="stable")  # ascending degree
    sdeg = din[order].astype(np.int64)
    rem_sel = np.zeros(npc, bool)
    t, lo, hi = T, 0, npc - 1
    for r in range(128, 0, -1):
        want = t / r
        j = int(np.searchsorted(sdeg[lo:hi + 1], want)) + lo
        j = min(j, hi)
        rem_sel[order[j]] = True
        t -= int(sdeg[j])
        # shrink window: remove chosen element by swapping bounds
        sdeg[j] = sdeg[hi]
        ohi = order[hi]
        order[hi] = order[j]
        order[j] = ohi
        sdeg_j = sdeg  # keep views consistent
        hi -= 1

    counts = np.zeros(NG, np.int64)
    sums = np.zeros(NG, np.int64)
    perm = np.empty(npc, np.int64)
    rem_ids = np.nonzero(rem_sel)[0]
    for i, c in enumerate(rem_ids):
        perm[c] = (NG - 1) * 128 + i
        sums[NG - 1] += int(din[c])
    counts[NG - 1] = 128

    BIG = 1 << 40
    main_order = np.argsort(-din, kind="stable")
    for c in main_order:
        if rem_sel[c]:
            continue
        key = sums + (counts >= 128) * BIG
        key[NG - 1] = BIG << 1
        g = int(np.argmin(key))
        assert counts[g] < 128
        perm[c] = g * 128 + counts[g]
        counts[g] += 1
        sums[g] += int(din[c])
    return perm, sums


def prep(inputs, cfg):
    """Host-side prep: sharding, edge sorting/grouping, layout packing.
    Returns (in_maps, S_g, perms) where S_g[g] = subtiles for col-group g."""
    x = np.asarray(inputs["x"], np.float32)
    ei = np.asarray(inputs["edge_index"])
    row = ei[0].astype(np.int64)
    col = ei[1].astype(np.int64)
    K = cfg.ncores

    deg = np.bincount(row, minlength=cfg.N).astype(np.float32)
    nd = np.clip(deg, 1.0, None) ** -0.5

    shard_r = row // cfg.NPCR
    rloc_all = (row - shard_r * cfg.NPCR).astype(np.int64)
    shard_c = col // cfg.NPCR
    cloc_all = (col - shard_c * cfg.NPCR).astype(np.int64)

    NG = cfg.NB
    # per-shard node permutation balancing incoming-edge load across groups;
    # main groups target cap_main edges (16 subtiles), excess goes to the
    # last group on every core
    cap_main = max(128, (cfg.E // (K * NG * 128)) * 128 + 120)
    perms = []
    for k in range(K):
        din = np.bincount(cloc_all[shard_c == k], minlength=cfg.NPC)
        perm, _ = _balance_cols(din, NG, cap_main)
        perms.append(perm)
    perms = np.stack(perms)  # [K, NPC] old local -> new local

    # remap global node id -> padded, permuted table row
    gidx_all = (shard_r * cfg.NPC + perms[shard_r, rloc_all]).astype(np.int32)

    # Edges whose source row lives in this core's own shard can be gathered
    # from the local (pre-AllGather) table copy, overlapping the collective.
    per_core = []
    cnt = np.zeros((K, NG), np.int64)
    lcnt = np.zeros((K, NG), np.int64)
    for k in range(K):
        m = shard_c == k
        ck = perms[k][cloc_all[m]]
        gk = gidx_all[m]
        is_loc = (shard_r[m] == k)
        lk = perms[k][rloc_all[m]]  # local table row (valid when is_loc)
        grp = ck >> 7
        cnt[k] = np.bincount(grp, minlength=NG)
        lcnt[k] = np.bincount(grp[is_loc], minlength=NG)
        per_core.append((ck, gk, grp, is_loc, lk))

    L_g = (lcnt.min(axis=0) // 128).astype(np.int64)     # all-local subtiles
    G_g = np.maximum(1, (np.max(cnt - 128 * L_g[None, :], axis=0) + 127) // 128)
    S_g = L_g + G_g
    S = int(S_g.sum())
    goff = np.concatenate([[0], np.cumsum(S_g)])[:-1] * 128  # edge-slot offset per group

    # pad edges gather this table row, which holds zeros (old pad row of shard 0)
    pad_row = int(perms[0][cfg.NPCR])

    w1t = np.ascontiguousarray(
        np.asarray(inputs["W1"], np.float32).T.reshape(cfg.FC, 128, cfg.H))
    w2t = np.ascontiguousarray(np.asarray(inputs["W2"], np.float32).T)
    b1 = np.asarray(inputs["b1"], np.float32).reshape(1, cfg.H)
    b2 = np.asarray(inputs["b2"], np.float32).reshape(1, cfg.C)
    eps1 = np.asarray(inputs["eps1"], np.float32).reshape(1, cfg.H)

    in_maps = []
    for k in range(K):
        ck, gk, grp, is_loc, lk = per_core[k]
        ne = len(ck)
        # rank of each local edge among its group's local edges
        o1 = np.argsort(grp, kind="stable")
        inv1 = np.empty(ne, np.int64)
        inv1[o1] = np.arange(ne)
        gs = grp[o1]
        ls = is_loc[o1].astype(np.int64)
        lcum = np.cumsum(ls) - ls  # locals before this position
        gstart = np.concatenate([[0], np.cumsum(np.bincount(gs, minlength=NG))])[:-1]
        lrank_sorted = lcum - lcum[gstart[gs]]
        lrank = lrank_sorted[inv1]
        # class 0 = fills the all-local subtiles (gathered from the local table)
        cls = np.where(is_loc & (lrank < 128 * L_g[grp]), 0, 1)
        order = np.argsort(grp * 2 + cls, kind="stable")
        grp_o = grp[order]
        starts = np.concatenate([[0], np.cumsum(cnt[k])])[:-1]
        pos = goff[grp_o] + (np.arange(ne) - starts[grp_o])
        vals = np.where(cls == 0, lk, gk)[order].astype(np.int32)
        gidx = np.full(S * 128, pad_row, np.int32)
        lpos = np.zeros(S * 128, np.float32)
        gidx[pos] = vals
        lpos[pos] = (ck[order] & 127).astype(np.float32)
        # [S,128] -> [128,S] partition-major
        gidx = np.ascontiguousarray(gidx.reshape(S, 128).T)
        lpos = np.ascontiguousarray(lpos.reshape(S, 128).T)

        xs = np.zeros((cfg.NPC, cfg.F), np.float32)
        xs[perms[k][:cfg.NPCR]] = x[k * cfg.NPCR:(k + 1) * cfg.NPCR]
        xtb = np.ascontiguousarray(
            xs.reshape(cfg.NB, 128, cfg.FC, 128).transpose(0, 2, 3, 1))

        nds = np.zeros(cfg.NPC, np.float32)
        nds[perms[k][:cfg.NPCR]] = nd[k * cfg.NPCR:(k + 1) * cfg.NPCR]
        ndp = np.ascontiguousarray(nds.reshape(cfg.NB, 128).T)

        in_maps.append({
            "xtb": xtb, "gidx": gidx, "lpos": lpos,
            "w1t": w1t, "w2t": w2t, "b1": b1, "b2": b2, "eps1": eps1,
            "ndp": ndp, "ndn": -ndp,
        })
    return in_maps, np.stack([L_g, G_g]), perms


def build(cfg, S_g):
    NB, FC, H, C = cfg.NB, cfg.FC, cfg.H, cfg.C
    NG = NB
    L_g, G_g = S_g[0], S_g[1]
    S = int(S_g.sum())
    nc = bacc.Bacc("TRN2", target_bir_lowering=False, debug=False,
                   num_devices=cfg.ncores)

    xtb = nc.dram_tensor("xtb", [NB, FC, 128, 128], F32, kind="ExternalInput")
    gidx_d = nc.dram_tensor("gidx", [128, S], I32, kind="ExternalInput")
    lpos_d = nc.dram_tensor("lpos", [128, S], F32, kind="ExternalInput")
    w1t_d = nc.dram_tensor("w1t", [FC, 128, H], F32, kind="ExternalInput")
    w2t_d = nc.dram_tensor("w2t", [H, C], F32, kind="ExternalInput")
    b1_d = nc.dram_tensor("b1", [1, H], F32, kind="ExternalInput")
    b2_d = nc.dram_tensor("b2", [1, C], F32, kind="ExternalInput")
    eps_d = nc.dram_tensor("eps1", [1, H], F32, kind="ExternalInput")
    ndp_d = nc.dram_tensor("ndp", [128, NB], F32, kind="ExternalInput")
    ndn_d = nc.dram_tensor("ndn", [128, NB], F32, kind="ExternalInput")

    lsm_o = nc.dram_tensor("lsm", [cfg.NPC, C], F32, kind="ExternalOutput")
    h2_o = nc.dram_tensor("h2o", [cfg.NPC, H], F32, kind="ExternalOutput")

    groups = [list(range(cfg.ncores))]

    with tile.TileContext(nc) as tc:
        with (
            tc.tile_pool(name="persist", bufs=1) as pp,
            tc.tile_pool(name="dram", bufs=1, space="DRAM") as dp,
            tc.tile_pool(name="xload", bufs=3) as xp,
            tc.tile_pool(name="ps", bufs=2, space="PSUM") as ps,
            tc.tile_pool(name="psb", bufs=2, space="PSUM") as psb,
            tc.tile_pool(name="gt", bufs=8) as gp,
            tc.tile_pool(name="sel", bufs=8) as sp,
            tc.tile_pool(name="sm", bufs=6) as sm,
        ):
            bounce1 = dp.tile([cfg.NPC, H], F32)
            bounce2 = dp.tile([cfg.NPC, H], F32)
            t1full = dp.tile([cfg.NFULL, H], F32, addr_space="Shared")
            t2full = dp.tile([cfg.NFULL, H], F32, addr_space="Shared")

            idx_sb = pp.tile([128, S], I32)
            lpos_sb = pp.tile([128, S], F32)
            w1_sb = pp.tile([128, FC * H], F32)
            w2_sb = pp.tile([H, C], F32)
            ndp_sb = pp.tile([128, NB], F32)
            ndn_sb = pp.tile([128, NB], F32)
            sraw = pp.tile([128, NB * H], F32)
            iota_f = pp.tile([128, 128], F32)
            ident = pp.tile([128, 128], F32)
            ones = pp.tile([1, 128], F32)
            scale128 = pp.tile([128, H], F32)
            b1b = pp.tile([128, H], F32)
            b2b = pp.tile([128, C], F32)
            eps_sb = pp.tile([1, H], F32)
            b1_sb = pp.tile([1, H], F32)
            b2_sb = pp.tile([1, C], F32)

            nc.sync.dma_start(idx_sb[:], gidx_d[:])
            nc.sync.dma_start(lpos_sb[:], lpos_d[:])
            nc.sync.dma_start(w1_sb[:].rearrange("k (c h) -> k c h", c=FC),
                              w1t_d[:].rearrange("c k h -> k c h"))
            nc.sync.dma_start(w2_sb[:], w2t_d[:])
            nc.sync.dma_start(ndp_sb[:], ndp_d[:])
            nc.sync.dma_start(ndn_sb[:], ndn_d[:])
            nc.sync.dma_start(eps_sb[:], eps_d[:])
            nc.sync.dma_start(b1_sb[:], b1_d[:])
            nc.sync.dma_start(b2_sb[:], b2_d[:])

            iota_i = sm.tile([128, 128], I32)
            nc.gpsimd.iota(iota_i[:], pattern=[[1, 128]], base=0,
                           channel_multiplier=0)
            nc.vector.tensor_copy(iota_f[:], iota_i[:])
            make_identity(nc, ident[:])
            nc.vector.memset(ones[:], 1.0)

            # scale = sigmoid(eps1)+1, broadcast to 128 partitions via K=1 matmul
            sig = sm.tile([1, H], F32)
            nc.scalar.activation(sig[:], eps_sb[:],
                                 mybir.ActivationFunctionType.Sigmoid)
            nc.vector.tensor_scalar_add(sig[:], sig[:], 1.0)
            pbc = psb.tile([128, H], F32, tag="tp")
            nc.tensor.matmul(pbc[:], lhsT=ones[:], rhs=sig[:], start=True, stop=True)
            nc.vector.tensor_copy(scale128[:], pbc[:])
            pbc2 = psb.tile([128, H], F32, tag="tp")
            nc.tensor.matmul(pbc2[:], lhsT=ones[:], rhs=b1_sb[:], start=True, stop=True)
            nc.vector.tensor_copy(b1b[:], pbc2[:])
            pbc3 = psb.tile([128, C], F32, tag="tp")
            nc.tensor.matmul(pbc3[:], lhsT=ones[:], rhs=b2_sb[:], start=True, stop=True)
            nc.vector.tensor_copy(b2b[:], pbc3[:])

            # ---- phase 1: raw/sraw + round-1 table (scaled by -nd[row]) ----
            for j in range(NB):
                xblk = xp.tile([128, FC * 128], F32, tag="x")
                nc.sync.dma_start(xblk[:].rearrange("k (c n) -> k c n", c=FC),
                                  xtb[j].rearrange("c k n -> k c n"))
                pm = ps.tile([128, H], F32, tag="mm")
                for c in range(FC):
                    nc.tensor.matmul(pm[:], lhsT=xblk[:, c * 128:(c + 1) * 128],
                                     rhs=w1_sb[:, c * H:(c + 1) * H],
                                     start=(c == 0), stop=(c == FC - 1))
                rawa = sm.tile([128, H], F32, tag="rawa")
                nc.vector.tensor_add(rawa[:], pm[:], b1b[:])
                rawb = sm.tile([128, H], F32, tag="rawb")
                nc.scalar.activation(rawb[:], rawa[:],
                                     mybir.ActivationFunctionType.Relu)
                nc.vector.tensor_mul(sraw[:, j * H:(j + 1) * H], rawb[:], scale128[:])
                tbl = sm.tile([128, H], F32, tag="tbl")
                nc.vector.tensor_scalar_mul(tbl[:], rawb[:], ndn_sb[:, j:j + 1])
                nc.sync.dma_start(bounce1[j * 128:(j + 1) * 128, :], tbl[:])

            lagg = pp.tile([128, NB * H], F32)

            def subtile(pm, s, src, start, stop):
                gt = gp.tile([128, H], F32, tag="gt", name="gt")
                nc.gpsimd.indirect_dma_start(
                    out=gt[:], out_offset=None, in_=src[:, :],
                    in_offset=bass.IndirectOffsetOnAxis(
                        ap=idx_sb[:, s:s + 1], axis=0))
                sel = sp.tile([128, 128], F32, tag="sel", name="sel")
                nc.vector.tensor_tensor(
                    out=sel[:],
                    in0=lpos_sb[:, s:s + 1].to_broadcast([128, 128]),
                    in1=iota_f[:], op=mybir.AluOpType.is_equal)
                nc.tensor.matmul(pm[:], lhsT=sel[:], rhs=gt[:],
                                 start=start, stop=stop)

            # edges from this core's own rows: gather from the local bounce
            # buffer while the AllGather is still in flight
            def local_phase(bounce):
                nc.vector.memset(lagg[:], 0.0)
                s_off = 0
                for g in range(NG):
                    n_l = int(L_g[g])
                    if n_l > 0:
                        pm = ps.tile([128, H], F32, tag="mm", name="pm")
                        for t in range(n_l):
                            subtile(pm, s_off + t, bounce, t == 0, t == n_l - 1)
                        nc.vector.tensor_copy(lagg[:, g * H:(g + 1) * H], pm[:])
                    s_off += n_l + int(G_g[g])

            def global_phase(tfull, emit):
                s_off = 0
                for g in range(NG):
                    n_l, n_t = int(L_g[g]), int(G_g[g])
                    pm = ps.tile([128, H], F32, tag="mm", name="pm")
                    for t in range(n_t):
                        subtile(pm, s_off + n_l + t, tfull, t == 0, t == n_t - 1)
                    s_off += n_l + n_t
                    emit(g, pm)

            nc.gpsimd.collective_compute(
                "AllGather", mybir.AluOpType.bypass, replica_groups=groups,
                ins=[bounce1[:].opt()], outs=[t1full[:].opt()])
            local_phase(bounce1)

            def emit_r1(g, pm):
                h1b = sm.tile([128, H], F32, tag="h1b")
                nc.vector.tensor_add(h1b[:], pm[:], lagg[:, g * H:(g + 1) * H])
                nc.vector.tensor_scalar_mul(h1b[:], h1b[:], ndp_sb[:, g:g + 1])
                nc.vector.tensor_add(h1b[:], h1b[:], sraw[:, g * H:(g + 1) * H])
                tbl = sm.tile([128, H], F32, tag="tbl")
                nc.vector.tensor_scalar_mul(tbl[:], h1b[:], ndn_sb[:, g:g + 1])
                nc.sync.dma_start(bounce2[g * 128:(g + 1) * 128, :], tbl[:])

            global_phase(t1full, emit_r1)

            nc.gpsimd.collective_compute(
                "AllGather", mybir.AluOpType.bypass, replica_groups=groups,
                ins=[bounce2[:].opt()], outs=[t2full[:].opt()])
            local_phase(bounce2)

            def emit_r2(g, pm):
                h2b = sm.tile([128, H], F32, tag="h2b")
                nc.vector.tensor_add(h2b[:], pm[:], lagg[:, g * H:(g + 1) * H])
                nc.vector.tensor_scalar_mul(h2b[:], h2b[:], ndp_sb[:, g:g + 1])
                nc.vector.tensor_add(h2b[:], h2b[:], sraw[:, g * H:(g + 1) * H])
                nc.sync.dma_start(h2_o[g * 128:(g + 1) * 128, :], h2b[:])
                # logits = h2 @ W2.T + b2 ; log_softmax rowwise
                tp = psb.tile([H, 128], F32, tag="tp")
                nc.tensor.transpose(out=tp[:], in_=h2b[:], identity=ident[:])
                h2t = sm.tile([H, 128], F32, tag="h2t")
                nc.vector.tensor_copy(h2t[:], tp[:])
                lp = psb.tile([128, C], F32, tag="lg")
                nc.tensor.matmul(lp[:], lhsT=h2t[:], rhs=w2_sb[:], start=True,
                                 stop=True)
                lg = sm.tile([128, C], F32, tag="lg2")
                nc.vector.tensor_add(lg[:], lp[:], b2b[:])
                nmx = sm.tile([128, 1], F32, tag="nmx")
                nc.vector.reduce_max(nmx[:], lg[:], axis=mybir.AxisListType.X,
                                     negate=True)
                ex = sm.tile([128, C], F32, tag="ex")
                nc.scalar.activation(ex[:], lg[:],
                                     mybir.ActivationFunctionType.Exp,
                                     bias=nmx[:])
                ssum = sm.tile([128, 1], F32, tag="ssum")
                nc.vector.reduce_sum(ssum[:], ex[:], axis=mybir.AxisListType.X)
                lns = sm.tile([128, 1], F32, tag="lns")
                nc.scalar.activation(lns[:], ssum[:],
                                     mybir.ActivationFunctionType.Ln)
                ls = sm.tile([128, C], F32, tag="ls")
                nc.vector.tensor_scalar(
                    out=ls[:], in0=lg[:], scalar1=nmx[:], scalar2=lns[:],
                    op0=mybir.AluOpType.add, op1=mybir.AluOpType.subtract)
                nc.sync.dma_start(lsm_o[g * 128:(g + 1) * 128, :], ls[:])

            global_phase(t2full, emit_r2)

    nc.compile()
    return nc


def assemble(results, cfg, perms):
    lsm = np.concatenate(
        [r["lsm"][perms[k][:cfg.NPCR]] for k, r in enumerate(results)], axis=0)
    h2 = np.concatenate(
        [r["h2o"][perms[k][:cfg.NPCR]] for k, r in enumerate(results)], axis=0)
    return lsm, h2


def kernel(**inputs):
    cfg = REAL
    in_maps, S_g, perms = prep(inputs, cfg)
    nc = build(cfg, S_g)
    res = bass_utils.run_bass_kernel_spmd(
        nc, in_maps, core_ids=list(range(cfg.ncores)))
    return assemble(res.results, cfg, perms)


# revision 31
# speedup vs baseline: 1.1659x; 1.1659x over previous
"""Trainium2 Bass kernel for the AFA-GNN message-passing network (8 NeuronCores).

Math (the tanh gate in the reference is dead code — overwritten with -1):
    deg = bincount(row);  nd = clip(deg,1)^-0.5;  g1_e = nd[row_e]*nd[col_e]
    raw  = relu(x @ W1.T + b1)
    sraw = (sigmoid(eps1)+1) * raw
    h1   = sraw - segsum_col(g1 * raw[row])
    h2   = sraw - segsum_col(g1 * h1[row])
    out  = (log_softmax(h2 @ W2.T + b2), h2)

Distribution: nodes (and their incoming edges, i.e. col-sharded) split over 8
cores.  Each core computes its node-shard of raw/sraw, pre-scales by -nd[row],
AllGathers the scaled table, then for each 128-col group gathers the table
rows of its edges (indirect DMA), builds a 0/1 col-selection matrix on the
vector engine, and segment-sums via matmul accumulation in PSUM.  The group
aggregate lands directly in SBUF node-major — no scatter is ever needed.
"""

import numpy as np

import concourse.bass as bass
import concourse.bacc as bacc
import concourse.tile as tile
from concourse import mybir, bass_utils
from concourse.masks import make_identity

F32 = mybir.dt.float32
BF16 = mybir.dt.bfloat16
I32 = mybir.dt.int32


class Cfg:
    def __init__(self, N, E, F, H, C, ncores=8):
        self.N, self.E, self.F, self.H, self.C = N, E, F, H, C
        self.ncores = ncores
        assert N % ncores == 0
        self.NPCR = N // ncores                 # real nodes per core
        self.NPC = ((self.NPCR + 127) // 128) * 128  # padded nodes per core
        # pad edges gather table row NPCR, which must be a zero pad row
        assert self.NPC > self.NPCR
        self.NB = self.NPC // 128               # node blocks == col groups
        self.FC = F // 128                      # feature chunks
        self.NFULL = ncores * self.NPC          # padded global table rows


REAL = Cfg(N=100000, E=1600000, F=512, H=64, C=40)


def _balance_cols(din, NG, cap_main):
    """Assign each local col to a group of exactly 128 cols.  Groups
    0..NG-2 are balanced (LPT) around cap_main edges; the core's excess
    beyond (NG-1)*cap_main is concentrated into group NG-1 so that the
    max-over-cores subtile count inflates a single group id only.
    Returns perm: old local col -> new local col."""
    npc = len(din)
    total = int(din.sum())

    # pick 128 cols for the remainder group with degree sum ~ T
    T = max(0, total - (NG - 1) * cap_main)
    order = np.argsort(din, kind="stable")  # ascending degree
    sdeg = din[order].astype(np.int64)
    rem_sel = np.zeros(npc, bool)
    t, lo, hi = T, 0, npc - 1
    for r in range(128, 0, -1):
        want = t / r
        j = int(np.searchsorted(sdeg[lo:hi + 1], want)) + lo
        j = min(j, hi)
        rem_sel[order[j]] = True
        t -= int(sdeg[j])
        # shrink window: remove chosen element by swapping bounds
        sdeg[j] = sdeg[hi]
        ohi = order[hi]
        order[hi] = order[j]
        order[j] = ohi
        sdeg_j = sdeg  # keep views consistent
        hi -= 1

    counts = np.zeros(NG, np.int64)
    sums = np.zeros(NG, np.int64)
    perm = np.empty(npc, np.int64)
    rem_ids = np.nonzero(rem_sel)[0]
    for i, c in enumerate(rem_ids):
        perm[c] = (NG - 1) * 128 + i
        sums[NG - 1] += int(din[c])
    counts[NG - 1] = 128

    BIG = 1 << 40
    main_order = np.argsort(-din, kind="stable")
    for c in main_order:
        if rem_sel[c]:
            continue
        key = sums + (counts >= 128) * BIG
        key[NG - 1] = BIG << 1
        g = int(np.argmin(key))
        assert counts[g] < 128
        perm[c] = g * 128 + counts[g]
        counts[g] += 1
        sums[g] += int(din[c])
    return perm, sums


def prep(inputs, cfg):
    """Host-side prep: sharding, edge sorting/grouping, layout packing.
    Returns (in_maps, S_g, perms) where S_g[g] = subtiles for col-group g."""
    x = np.asarray(inputs["x"], np.float32)
    ei = np.asarray(inputs["edge_index"])
    row = ei[0].astype(np.int64)
    col = ei[1].astype(np.int64)
    K = cfg.ncores

    deg = np.bincount(row, minlength=cfg.N).astype(np.float32)
    nd = np.clip(deg, 1.0, None) ** -0.5

    shard_r = row // cfg.NPCR
    rloc_all = (row - shard_r * cfg.NPCR).astype(np.int64)
    shard_c = col // cfg.NPCR
    cloc_all = (col - shard_c * cfg.NPCR).astype(np.int64)

    NG = cfg.NB
    # per-shard node permutation balancing incoming-edge load across groups;
    # main groups target cap_main edges (16 subtiles), excess goes to the
    # last group on every core
    cap_main = max(128, (cfg.E // (K * NG * 128)) * 128 + 120)
    perms = []
    for k in range(K):
        din = np.bincount(cloc_all[shard_c == k], minlength=cfg.NPC)
        perm, _ = _balance_cols(din, NG, cap_main)
        perms.append(perm)
    perms = np.stack(perms)  # [K, NPC] old local -> new local

    # remap global node id -> padded, permuted table row
    gidx_all = (shard_r * cfg.NPC + perms[shard_r, rloc_all]).astype(np.int32)

    # Edges whose source row lives in this core's own shard can be gathered
    # from the local (pre-AllGather) table copy, overlapping the collective.
    per_core = []
    cnt = np.zeros((K, NG), np.int64)
    lcnt = np.zeros((K, NG), np.int64)
    for k in range(K):
        m = shard_c == k
        ck = perms[k][cloc_all[m]]
        gk = gidx_all[m]
        is_loc = (shard_r[m] == k)
        lk = perms[k][rloc_all[m]]  # local table row (valid when is_loc)
        grp = ck >> 7
        cnt[k] = np.bincount(grp, minlength=NG)
        lcnt[k] = np.bincount(grp[is_loc], minlength=NG)
        per_core.append((ck, gk, grp, is_loc, lk))

    L_g = (lcnt.min(axis=0) // 128).astype(np.int64)     # all-local subtiles
    G_g = np.maximum(1, (np.max(cnt - 128 * L_g[None, :], axis=0) + 127) // 128)
    S_g = L_g + G_g
    S = int(S_g.sum())
    goff = np.concatenate([[0], np.cumsum(S_g)])[:-1] * 128  # edge-slot offset per group

    # pad edges gather this table row, which holds zeros (old pad row of shard 0)
    pad_row = int(perms[0][cfg.NPCR])

    w1t = np.ascontiguousarray(
        np.asarray(inputs["W1"], np.float32).T.reshape(cfg.FC, 128, cfg.H))
    w2t = np.ascontiguousarray(np.asarray(inputs["W2"], np.float32).T)
    b1 = np.asarray(inputs["b1"], np.float32).reshape(1, cfg.H)
    b2 = np.asarray(inputs["b2"], np.float32).reshape(1, cfg.C)
    eps1 = np.asarray(inputs["eps1"], np.float32).reshape(1, cfg.H)

    in_maps = []
    for k in range(K):
        ck, gk, grp, is_loc, lk = per_core[k]
        ne = len(ck)
        # rank of each local edge among its group's local edges
        o1 = np.argsort(grp, kind="stable")
        inv1 = np.empty(ne, np.int64)
        inv1[o1] = np.arange(ne)
        gs = grp[o1]
        ls = is_loc[o1].astype(np.int64)
        lcum = np.cumsum(ls) - ls  # locals before this position
        gstart = np.concatenate([[0], np.cumsum(np.bincount(gs, minlength=NG))])[:-1]
        lrank_sorted = lcum - lcum[gstart[gs]]
        lrank = lrank_sorted[inv1]
        # class 0 = fills the all-local subtiles (gathered from the local table)
        cls = np.where(is_loc & (lrank < 128 * L_g[grp]), 0, 1)
        order = np.argsort(grp * 2 + cls, kind="stable")
        grp_o = grp[order]
        starts = np.concatenate([[0], np.cumsum(cnt[k])])[:-1]
        pos = goff[grp_o] + (np.arange(ne) - starts[grp_o])
        vals = np.where(cls == 0, lk, gk)[order].astype(np.int32)
        gidx = np.full(S * 128, pad_row, np.int32)
        lpos = np.zeros(S * 128, np.float32)
        gidx[pos] = vals
        lpos[pos] = (ck[order] & 127).astype(np.float32)
        # [S,128] -> [128,S] partition-major
        gidx = np.ascontiguousarray(gidx.reshape(S, 128).T)
        lpos = np.ascontiguousarray(lpos.reshape(S, 128).T)

        xs = np.zeros((cfg.NPC, cfg.F), np.float32)
        xs[perms[k][:cfg.NPCR]] = x[k * cfg.NPCR:(k + 1) * cfg.NPCR]
        # [block j][feat-in-chunk kk][chunk c][node n]: per SBUF partition (kk)
        # the whole FC*128 free dim is contiguous in DRAM
        xtb = np.ascontiguousarray(
            xs.reshape(cfg.NB, 128, cfg.FC, 128).transpose(0, 3, 2, 1))

        nds = np.zeros(cfg.NPC, np.float32)
        nds[perms[k][:cfg.NPCR]] = nd[k * cfg.NPCR:(k + 1) * cfg.NPCR]
        ndp = np.ascontiguousarray(nds.reshape(cfg.NB, 128).T)

        in_maps.append({
            "xtb": xtb, "gidx": gidx, "lpos": lpos,
            "w1t": w1t, "w2t": w2t, "b1": b1, "b2": b2, "eps1": eps1,
            "ndp": ndp, "ndn": -ndp,
        })
    return in_maps, np.stack([L_g, G_g]), perms


def build(cfg, S_g):
    NB, FC, H, C = cfg.NB, cfg.FC, cfg.H, cfg.C
    NG = NB
    L_g, G_g = S_g[0], S_g[1]
    S = int(S_g.sum())
    nc = bacc.Bacc("TRN2", target_bir_lowering=False, debug=False,
                   num_devices=cfg.ncores)

    xtb = nc.dram_tensor("xtb", [NB, 128, FC, 128], F32, kind="ExternalInput")
    gidx_d = nc.dram_tensor("gidx", [128, S], I32, kind="ExternalInput")
    lpos_d = nc.dram_tensor("lpos", [128, S], F32, kind="ExternalInput")
    w1t_d = nc.dram_tensor("w1t", [FC, 128, H], F32, kind="ExternalInput")
    w2t_d = nc.dram_tensor("w2t", [H, C], F32, kind="ExternalInput")
    b1_d = nc.dram_tensor("b1", [1, H], F32, kind="ExternalInput")
    b2_d = nc.dram_tensor("b2", [1, C], F32, kind="ExternalInput")
    eps_d = nc.dram_tensor("eps1", [1, H], F32, kind="ExternalInput")
    ndp_d = nc.dram_tensor("ndp", [128, NB], F32, kind="ExternalInput")
    ndn_d = nc.dram_tensor("ndn", [128, NB], F32, kind="ExternalInput")

    lsm_o = nc.dram_tensor("lsm", [cfg.NPC, C], F32, kind="ExternalOutput")
    h2_o = nc.dram_tensor("h2o", [cfg.NPC, H], F32, kind="ExternalOutput")

    groups = [list(range(cfg.ncores))]

    with tile.TileContext(nc) as tc:
        with (
            tc.tile_pool(name="persist", bufs=1) as pp,
            tc.tile_pool(name="dram", bufs=1, space="DRAM") as dp,
            tc.tile_pool(name="xload", bufs=3) as xp,
            tc.tile_pool(name="ps", bufs=2, space="PSUM") as ps,
            tc.tile_pool(name="psb", bufs=2, space="PSUM") as psb,
            tc.tile_pool(name="gt", bufs=8) as gp,
            tc.tile_pool(name="sel", bufs=8) as sp,
            tc.tile_pool(name="sm", bufs=6) as sm,
        ):
            bounce1 = dp.tile([cfg.NPC, H], F32)
            bounce2 = dp.tile([cfg.NPC, H], F32)
            t1full = dp.tile([cfg.NFULL, H], F32, addr_space="Shared")
            t2full = dp.tile([cfg.NFULL, H], F32, addr_space="Shared")

            idx_sb = pp.tile([128, S], I32)
            lpos_sb = pp.tile([128, S], F32)
            w1_sb = pp.tile([128, FC * H], F32)
            w2_sb = pp.tile([H, C], F32)
            ndp_sb = pp.tile([128, NB], F32)
            ndn_sb = pp.tile([128, NB], F32)
            sraw = pp.tile([128, NB * H], F32)
            iota_f = pp.tile([128, 128], F32)
            ident = pp.tile([128, 128], F32)
            ones = pp.tile([1, 128], F32)
            scale128 = pp.tile([128, H], F32)
            b1b = pp.tile([128, H], F32)
            b2b = pp.tile([128, C], F32)
            eps_sb = pp.tile([1, H], F32)
            b1_sb = pp.tile([1, H], F32)
            b2_sb = pp.tile([1, C], F32)

            nc.sync.dma_start(idx_sb[:], gidx_d[:])
            nc.sync.dma_start(lpos_sb[:], lpos_d[:])
            nc.sync.dma_start(w1_sb[:].rearrange("k (c h) -> k c h", c=FC),
                              w1t_d[:].rearrange("c k h -> k c h"))
            nc.sync.dma_start(w2_sb[:], w2t_d[:])
            nc.sync.dma_start(ndp_sb[:], ndp_d[:])
            nc.sync.dma_start(ndn_sb[:], ndn_d[:])
            nc.sync.dma_start(eps_sb[:], eps_d[:])
            nc.sync.dma_start(b1_sb[:], b1_d[:])
            nc.sync.dma_start(b2_sb[:], b2_d[:])

            iota_i = sm.tile([128, 128], I32)
            nc.gpsimd.iota(iota_i[:], pattern=[[1, 128]], base=0,
                           channel_multiplier=0)
            nc.vector.tensor_copy(iota_f[:], iota_i[:])
            make_identity(nc, ident[:])
            nc.vector.memset(ones[:], 1.0)

            # scale = sigmoid(eps1)+1, broadcast to 128 partitions via K=1 matmul
            sig = sm.tile([1, H], F32)
            nc.scalar.activation(sig[:], eps_sb[:],
                                 mybir.ActivationFunctionType.Sigmoid)
            nc.vector.tensor_scalar_add(sig[:], sig[:], 1.0)
            pbc = psb.tile([128, H], F32, tag="tp")
            nc.tensor.matmul(pbc[:], lhsT=ones[:], rhs=sig[:], start=True, stop=True)
            nc.vector.tensor_copy(scale128[:], pbc[:])
            pbc2 = psb.tile([128, H], F32, tag="tp")
            nc.tensor.matmul(pbc2[:], lhsT=ones[:], rhs=b1_sb[:], start=True, stop=True)
            nc.vector.tensor_copy(b1b[:], pbc2[:])
            pbc3 = psb.tile([128, C], F32, tag="tp")
            nc.tensor.matmul(pbc3[:], lhsT=ones[:], rhs=b2_sb[:], start=True, stop=True)
            nc.vector.tensor_copy(b2b[:], pbc3[:])

            # ---- phase 1: raw/sraw + round-1 table (scaled by -nd[row]) ----
            for j in range(NB):
                xblk = xp.tile([128, FC * 128], F32, tag="x")
                nc.sync.dma_start(xblk[:].rearrange("k (c n) -> k c n", c=FC),
                                  xtb[j])
                pm = ps.tile([128, H], F32, tag="mm")
                for c in range(FC):
                    nc.tensor.matmul(pm[:], lhsT=xblk[:, c * 128:(c + 1) * 128],
                                     rhs=w1_sb[:, c * H:(c + 1) * H],
                                     start=(c == 0), stop=(c == FC - 1))
                rawa = sm.tile([128, H], F32, tag="rawa")
                nc.vector.tensor_add(rawa[:], pm[:], b1b[:])
                rawb = sm.tile([128, H], F32, tag="rawb")
                nc.scalar.activation(rawb[:], rawa[:],
                                     mybir.ActivationFunctionType.Relu)
                nc.vector.tensor_mul(sraw[:, j * H:(j + 1) * H], rawb[:], scale128[:])
                tbl = sm.tile([128, H], F32, tag="tbl")
                nc.vector.tensor_scalar_mul(tbl[:], rawb[:], ndn_sb[:, j:j + 1])
                nc.sync.dma_start(bounce1[j * 128:(j + 1) * 128, :], tbl[:])

            lagg = pp.tile([128, NB * H], F32)

            def subtile(pm, s, src, start, stop):
                gt = gp.tile([128, H], F32, tag="gt", name="gt")
                nc.gpsimd.indirect_dma_start(
                    out=gt[:], out_offset=None, in_=src[:, :],
                    in_offset=bass.IndirectOffsetOnAxis(
                        ap=idx_sb[:, s:s + 1], axis=0))
                sel = sp.tile([128, 128], F32, tag="sel", name="sel")
                nc.vector.tensor_tensor(
                    out=sel[:],
                    in0=lpos_sb[:, s:s + 1].to_broadcast([128, 128]),
                    in1=iota_f[:], op=mybir.AluOpType.is_equal)
                nc.tensor.matmul(pm[:], lhsT=sel[:], rhs=gt[:],
                                 start=start, stop=stop)

            # edges from this core's own rows: gather from the local bounce
            # buffer while the AllGather is still in flight
            def local_phase(bounce):
                nc.vector.memset(lagg[:], 0.0)
                s_off = 0
                for g in range(NG):
                    n_l = int(L_g[g])
                    if n_l > 0:
                        pm = ps.tile([128, H], F32, tag="mm", name="pm")
                        for t in range(n_l):
                            subtile(pm, s_off + t, bounce, t == 0, t == n_l - 1)
                        nc.vector.tensor_copy(lagg[:, g * H:(g + 1) * H], pm[:])
                    s_off += n_l + int(G_g[g])

            def global_phase(tfull, emit):
                s_off = 0
                for g in range(NG):
                    n_l, n_t = int(L_g[g]), int(G_g[g])
                    pm = ps.tile([128, H], F32, tag="mm", name="pm")
                    for t in range(n_t):
                        subtile(pm, s_off + n_l + t, tfull, t == 0, t == n_t - 1)
                    s_off += n_l + n_t
                    emit(g, pm)

            nc.gpsimd.collective_compute(
                "AllGather", mybir.AluOpType.bypass, replica_groups=groups,
                ins=[bounce1[:].opt()], outs=[t1full[:].opt()])
            local_phase(bounce1)

            def emit_r1(g, pm):
                h1b = sm.tile([128, H], F32, tag="h1b")
                nc.vector.tensor_add(h1b[:], pm[:], lagg[:, g * H:(g + 1) * H])
                nc.vector.tensor_scalar_mul(h1b[:], h1b[:], ndp_sb[:, g:g + 1])
                nc.vector.tensor_add(h1b[:], h1b[:], sraw[:, g * H:(g + 1) * H])
                tbl = sm.tile([128, H], F32, tag="tbl")
                nc.vector.tensor_scalar_mul(tbl[:], h1b[:], ndn_sb[:, g:g + 1])
                nc.sync.dma_start(bounce2[g * 128:(g + 1) * 128, :], tbl[:])

            global_phase(t1full, emit_r1)

            nc.gpsimd.collective_compute(
                "AllGather", mybir.AluOpType.bypass, replica_groups=groups,
                ins=[bounce2[:].opt()], outs=[t2full[:].opt()])
            local_phase(bounce2)

            def emit_r2(g, pm):
                h2b = sm.tile([128, H], F32, tag="h2b")
                nc.vector.tensor_add(h2b[:], pm[:], lagg[:, g * H:(g + 1) * H])
                nc.vector.tensor_scalar_mul(h2b[:], h2b[:], ndp_sb[:, g:g + 1])
                nc.vector.tensor_add(h2b[:], h2b[:], sraw[:, g * H:(g + 1) * H])
                nc.sync.dma_start(h2_o[g * 128:(g + 1) * 128, :], h2b[:])
                # logits = h2 @ W2.T + b2 ; log_softmax rowwise
                tp = psb.tile([H, 128], F32, tag="tp")
                nc.tensor.transpose(out=tp[:], in_=h2b[:], identity=ident[:])
                h2t = sm.tile([H, 128], F32, tag="h2t")
                nc.vector.tensor_copy(h2t[:], tp[:])
                lp = psb.tile([128, C], F32, tag="lg")
                nc.tensor.matmul(lp[:], lhsT=h2t[:], rhs=w2_sb[:], start=True,
                                 stop=True)
                lg = sm.tile([128, C], F32, tag="lg2")
                nc.vector.tensor_add(lg[:], lp[:], b2b[:])
                nmx = sm.tile([128, 1], F32, tag="nmx")
                nc.vector.reduce_max(nmx[:], lg[:], axis=mybir.AxisListType.X,
                                     negate=True)
                ex = sm.tile([128, C], F32, tag="ex")
                nc.scalar.activation(ex[:], lg[:],
                                     mybir.ActivationFunctionType.Exp,
                                     bias=nmx[:])
                ssum = sm.tile([128, 1], F32, tag="ssum")
                nc.vector.reduce_sum(ssum[:], ex[:], axis=mybir.AxisListType.X)
                lns = sm.tile([128, 1], F32, tag="lns")
                nc.scalar.activation(lns[:], ssum[:],
                                     mybir.ActivationFunctionType.Ln)
                ls = sm.tile([128, C], F32, tag="ls")
                nc.vector.tensor_scalar(
                    out=ls[:], in0=lg[:], scalar1=nmx[:], scalar2=lns[:],
                    op0=mybir.AluOpType.add, op1=mybir.AluOpType.subtract)
                nc.sync.dma_start(lsm_o[g * 128:(g + 1) * 128, :], ls[:])

            global_phase(t2full, emit_r2)

    nc.compile()
    return nc


def assemble(results, cfg, perms):
    lsm = np.concatenate(
        [r["lsm"][perms[k][:cfg.NPCR]] for k, r in enumerate(results)], axis=0)
    h2 = np.concatenate(
        [r["h2o"][perms[k][:cfg.NPCR]] for k, r in enumerate(results)], axis=0)
    return lsm, h2


def kernel(**inputs):
    cfg = REAL
    in_maps, S_g, perms = prep(inputs, cfg)
    nc = build(cfg, S_g)
    res = bass_utils.run_bass_kernel_spmd(
        nc, in_maps, core_ids=list(range(cfg.ncores)))
    return assemble(res.results, cfg, perms)


# revision 32
# speedup vs baseline: 1.1731x; 1.0062x over previous
"""Trainium2 Bass kernel for the AFA-GNN message-passing network (8 NeuronCores).

Math (the tanh gate in the reference is dead code — overwritten with -1):
    deg = bincount(row);  nd = clip(deg,1)^-0.5;  g1_e = nd[row_e]*nd[col_e]
    raw  = relu(x @ W1.T + b1)
    sraw = (sigmoid(eps1)+1) * raw
    h1   = sraw - segsum_col(g1 * raw[row])
    h2   = sraw - segsum_col(g1 * h1[row])
    out  = (log_softmax(h2 @ W2.T + b2), h2)

Distribution: nodes (and their incoming edges, i.e. col-sharded) split over 8
cores.  Each core computes its node-shard of raw/sraw, pre-scales by -nd[row],
AllGathers the scaled table, then for each 128-col group gathers the table
rows of its edges (indirect DMA), builds a 0/1 col-selection matrix on the
vector engine, and segment-sums via matmul accumulation in PSUM.  The group
aggregate lands directly in SBUF node-major — no scatter is ever needed.
"""

import numpy as np

import concourse.bass as bass
import concourse.bacc as bacc
import concourse.tile as tile
from concourse import mybir, bass_utils
from concourse.masks import make_identity

F32 = mybir.dt.float32
I32 = mybir.dt.int32


class Cfg:
    def __init__(self, N, E, F, H, C, ncores=8):
        self.N, self.E, self.F, self.H, self.C = N, E, F, H, C
        self.ncores = ncores
        assert N % ncores == 0
        self.NPCR = N // ncores                 # real nodes per core
        self.NPC = ((self.NPCR + 127) // 128) * 128  # padded nodes per core
        # pad edges gather table row NPCR, which must be a zero pad row
        assert self.NPC > self.NPCR
        self.NB = self.NPC // 128               # node blocks == col groups
        self.FC = F // 128                      # feature chunks
        self.NFULL = ncores * self.NPC          # padded global table rows


REAL = Cfg(N=100000, E=1600000, F=512, H=64, C=40)


def _balance_cols(din, NG, cap_main):
    """Assign each local col to a group of exactly 128 cols.  Groups
    0..NG-2 are balanced (LPT) around cap_main edges; the core's excess
    beyond (NG-1)*cap_main is concentrated into group NG-1 so that the
    max-over-cores subtile count inflates a single group id only.
    Returns perm: old local col -> new local col."""
    npc = len(din)
    total = int(din.sum())

    # pick 128 cols for the remainder group with degree sum ~ T
    T = max(0, total - (NG - 1) * cap_main)
    order = np.argsort(din, kind="stable")  # ascending degree
    sdeg = din[order].astype(np.int64)
    rem_sel = np.zeros(npc, bool)
    t, lo, hi = T, 0, npc - 1
    for r in range(128, 0, -1):
        want = t / r
        j = int(np.searchsorted(sdeg[lo:hi + 1], want)) + lo
        j = min(j, hi)
        rem_sel[order[j]] = True
        t -= int(sdeg[j])
        # remove chosen element by swapping it past the shrinking window bound
        sdeg[j] = sdeg[hi]
        ohi = order[hi]
        order[hi] = order[j]
        order[j] = ohi
        hi -= 1

    counts = np.zeros(NG, np.int64)
    sums = np.zeros(NG, np.int64)
    perm = np.empty(npc, np.int64)
    rem_ids = np.nonzero(rem_sel)[0]
    for i, c in enumerate(rem_ids):
        perm[c] = (NG - 1) * 128 + i
        sums[NG - 1] += int(din[c])
    counts[NG - 1] = 128

    BIG = 1 << 40
    main_order = np.argsort(-din, kind="stable")
    for c in main_order:
        if rem_sel[c]:
            continue
        key = sums + (counts >= 128) * BIG
        key[NG - 1] = BIG << 1
        g = int(np.argmin(key))
        assert counts[g] < 128
        perm[c] = g * 128 + counts[g]
        counts[g] += 1
        sums[g] += int(din[c])
    return perm, sums


def prep(inputs, cfg):
    """Host-side prep: sharding, edge sorting/grouping, layout packing.
    Returns (in_maps, S_g, perms) where S_g[g] = subtiles for col-group g."""
    x = np.asarray(inputs["x"], np.float32)
    ei = np.asarray(inputs["edge_index"])
    row = ei[0].astype(np.int64)
    col = ei[1].astype(np.int64)
    K = cfg.ncores

    deg = np.bincount(row, minlength=cfg.N).astype(np.float32)
    nd = np.clip(deg, 1.0, None) ** -0.5

    shard_r = row // cfg.NPCR
    rloc_all = (row - shard_r * cfg.NPCR).astype(np.int64)
    shard_c = col // cfg.NPCR
    cloc_all = (col - shard_c * cfg.NPCR).astype(np.int64)

    NG = cfg.NB
    # per-shard node permutation balancing incoming-edge load across groups;
    # main groups target cap_main edges (16 subtiles), excess goes to the
    # last group on every core
    cap_main = max(128, (cfg.E // (K * NG * 128)) * 128 + 120)
    perms = []
    for k in range(K):
        din = np.bincount(cloc_all[shard_c == k], minlength=cfg.NPC)
        perm, _ = _balance_cols(din, NG, cap_main)
        perms.append(perm)
    perms = np.stack(perms)  # [K, NPC] old local -> new local

    # remap global node id -> padded, permuted table row
    gidx_all = (shard_r * cfg.NPC + perms[shard_r, rloc_all]).astype(np.int32)

    # Edges whose source row lives in this core's own shard can be gathered
    # from the local (pre-AllGather) table copy, overlapping the collective.
    per_core = []
    cnt = np.zeros((K, NG), np.int64)
    lcnt = np.zeros((K, NG), np.int64)
    for k in range(K):
        m = shard_c == k
        ck = perms[k][cloc_all[m]]
        gk = gidx_all[m]
        is_loc = (shard_r[m] == k)
        lk = perms[k][rloc_all[m]]  # local table row (valid when is_loc)
        grp = ck >> 7
        cnt[k] = np.bincount(grp, minlength=NG)
        lcnt[k] = np.bincount(grp[is_loc], minlength=NG)
        per_core.append((ck, gk, grp, is_loc, lk))

    L_g = (lcnt.min(axis=0) // 128).astype(np.int64)     # all-local subtiles
    G_g = np.maximum(1, (np.max(cnt - 128 * L_g[None, :], axis=0) + 127) // 128)
    S_g = L_g + G_g
    S = int(S_g.sum())
    goff = np.concatenate([[0], np.cumsum(S_g)])[:-1] * 128  # edge-slot offset per group

    # pad edges gather this table row, which holds zeros (old pad row of shard 0)
    pad_row = int(perms[0][cfg.NPCR])

    w1t = np.ascontiguousarray(
        np.asarray(inputs["W1"], np.float32).T.reshape(cfg.FC, 128, cfg.H))
    w2t = np.ascontiguousarray(np.asarray(inputs["W2"], np.float32).T)
    b1 = np.asarray(inputs["b1"], np.float32).reshape(1, cfg.H)
    b2 = np.asarray(inputs["b2"], np.float32).reshape(1, cfg.C)
    eps1 = np.asarray(inputs["eps1"], np.float32).reshape(1, cfg.H)

    in_maps = []
    for k in range(K):
        ck, gk, grp, is_loc, lk = per_core[k]
        ne = len(ck)
        # rank of each local edge among its group's local edges
        o1 = np.argsort(grp, kind="stable")
        inv1 = np.empty(ne, np.int64)
        inv1[o1] = np.arange(ne)
        gs = grp[o1]
        ls = is_loc[o1].astype(np.int64)
        lcum = np.cumsum(ls) - ls  # locals before this position
        gstart = np.concatenate([[0], np.cumsum(np.bincount(gs, minlength=NG))])[:-1]
        lrank_sorted = lcum - lcum[gstart[gs]]
        lrank = lrank_sorted[inv1]
        # class 0 = fills the all-local subtiles (gathered from the local table)
        cls = np.where(is_loc & (lrank < 128 * L_g[grp]), 0, 1)
        order = np.argsort(grp * 2 + cls, kind="stable")
        grp_o = grp[order]
        starts = np.concatenate([[0], np.cumsum(cnt[k])])[:-1]
        pos = goff[grp_o] + (np.arange(ne) - starts[grp_o])
        vals = np.where(cls == 0, lk, gk)[order].astype(np.int32)
        gidx = np.full(S * 128, pad_row, np.int32)
        lpos = np.zeros(S * 128, np.float32)
        gidx[pos] = vals
        lpos[pos] = (ck[order] & 127).astype(np.float32)
        # [S,128] -> [128,S] partition-major
        gidx = np.ascontiguousarray(gidx.reshape(S, 128).T)
        lpos = np.ascontiguousarray(lpos.reshape(S, 128).T)

        xs = np.zeros((cfg.NPC, cfg.F), np.float32)
        xs[perms[k][:cfg.NPCR]] = x[k * cfg.NPCR:(k + 1) * cfg.NPCR]
        # [block j][feat-in-chunk kk][chunk c][node n]: per SBUF partition (kk)
        # the whole FC*128 free dim is contiguous in DRAM
        xtb = np.ascontiguousarray(
            xs.reshape(cfg.NB, 128, cfg.FC, 128).transpose(0, 3, 2, 1))

        nds = np.zeros(cfg.NPC, np.float32)
        nds[perms[k][:cfg.NPCR]] = nd[k * cfg.NPCR:(k + 1) * cfg.NPCR]
        ndp = np.ascontiguousarray(nds.reshape(cfg.NB, 128).T)

        in_maps.append({
            "xtb": xtb, "gidx": gidx, "lpos": lpos,
            "w1t": w1t, "w2t": w2t, "b1": b1, "b2": b2, "eps1": eps1,
            "ndp": ndp, "ndn": -ndp,
        })
    return in_maps, np.stack([L_g, G_g]), perms


def build(cfg, S_g):
    NB, FC, H, C = cfg.NB, cfg.FC, cfg.H, cfg.C
    NG = NB
    L_g, G_g = S_g[0], S_g[1]
    S = int(S_g.sum())
    nc = bacc.Bacc("TRN2", target_bir_lowering=False, debug=False,
                   num_devices=cfg.ncores)

    xtb = nc.dram_tensor("xtb", [NB, 128, FC, 128], F32, kind="ExternalInput")
    gidx_d = nc.dram_tensor("gidx", [128, S], I32, kind="ExternalInput")
    lpos_d = nc.dram_tensor("lpos", [128, S], F32, kind="ExternalInput")
    w1t_d = nc.dram_tensor("w1t", [FC, 128, H], F32, kind="ExternalInput")
    w2t_d = nc.dram_tensor("w2t", [H, C], F32, kind="ExternalInput")
    b1_d = nc.dram_tensor("b1", [1, H], F32, kind="ExternalInput")
    b2_d = nc.dram_tensor("b2", [1, C], F32, kind="ExternalInput")
    eps_d = nc.dram_tensor("eps1", [1, H], F32, kind="ExternalInput")
    ndp_d = nc.dram_tensor("ndp", [128, NB], F32, kind="ExternalInput")
    ndn_d = nc.dram_tensor("ndn", [128, NB], F32, kind="ExternalInput")

    lsm_o = nc.dram_tensor("lsm", [cfg.NPC, C], F32, kind="ExternalOutput")
    h2_o = nc.dram_tensor("h2o", [cfg.NPC, H], F32, kind="ExternalOutput")

    groups = [list(range(cfg.ncores))]

    with tile.TileContext(nc) as tc:
        with (
            tc.tile_pool(name="persist", bufs=1) as pp,
            tc.tile_pool(name="dram", bufs=1, space="DRAM") as dp,
            tc.tile_pool(name="xload", bufs=3) as xp,
            tc.tile_pool(name="ps", bufs=2, space="PSUM") as ps,
            tc.tile_pool(name="psb", bufs=2, space="PSUM") as psb,
            tc.tile_pool(name="gt", bufs=8) as gp,
            tc.tile_pool(name="sel", bufs=8) as sp,
            tc.tile_pool(name="sm", bufs=6) as sm,
        ):
            bounce1 = dp.tile([cfg.NPC, H], F32)
            bounce2 = dp.tile([cfg.NPC, H], F32)
            t1full = dp.tile([cfg.NFULL, H], F32, addr_space="Shared")
            t2full = dp.tile([cfg.NFULL, H], F32, addr_space="Shared")

            idx_sb = pp.tile([128, S], I32)
            lpos_sb = pp.tile([128, S], F32)
            w1_sb = pp.tile([128, FC * H], F32)
            w2_sb = pp.tile([H, C], F32)
            ndp_sb = pp.tile([128, NB], F32)
            ndn_sb = pp.tile([128, NB], F32)
            sraw = pp.tile([128, NB * H], F32)
            iota_f = pp.tile([128, 128], F32)
            ident = pp.tile([128, 128], F32)
            ones = pp.tile([1, 128], F32)
            scale128 = pp.tile([128, H], F32)
            b1b = pp.tile([128, H], F32)
            b2b = pp.tile([128, C], F32)
            eps_sb = pp.tile([1, H], F32)
            b1_sb = pp.tile([1, H], F32)
            b2_sb = pp.tile([1, C], F32)

            nc.sync.dma_start(idx_sb[:], gidx_d[:])
            nc.sync.dma_start(lpos_sb[:], lpos_d[:])
            nc.sync.dma_start(w1_sb[:].rearrange("k (c h) -> k c h", c=FC),
                              w1t_d[:].rearrange("c k h -> k c h"))
            nc.sync.dma_start(w2_sb[:], w2t_d[:])
            nc.sync.dma_start(ndp_sb[:], ndp_d[:])
            nc.sync.dma_start(ndn_sb[:], ndn_d[:])
            nc.sync.dma_start(eps_sb[:], eps_d[:])
            nc.sync.dma_start(b1_sb[:], b1_d[:])
            nc.sync.dma_start(b2_sb[:], b2_d[:])

            iota_i = sm.tile([128, 128], I32)
            nc.gpsimd.iota(iota_i[:], pattern=[[1, 128]], base=0,
                           channel_multiplier=0)
            nc.vector.tensor_copy(iota_f[:], iota_i[:])
            make_identity(nc, ident[:])
            nc.vector.memset(ones[:], 1.0)

            # scale = sigmoid(eps1)+1, broadcast to 128 partitions via K=1 matmul
            sig = sm.tile([1, H], F32)
            nc.scalar.activation(sig[:], eps_sb[:],
                                 mybir.ActivationFunctionType.Sigmoid)
            nc.vector.tensor_scalar_add(sig[:], sig[:], 1.0)
            pbc = psb.tile([128, H], F32, tag="tp")
            nc.tensor.matmul(pbc[:], lhsT=ones[:], rhs=sig[:], start=True, stop=True)
            nc.vector.tensor_copy(scale128[:], pbc[:])
            pbc2 = psb.tile([128, H], F32, tag="tp")
            nc.tensor.matmul(pbc2[:], lhsT=ones[:], rhs=b1_sb[:], start=True, stop=True)
            nc.vector.tensor_copy(b1b[:], pbc2[:])
            pbc3 = psb.tile([128, C], F32, tag="tp")
            nc.tensor.matmul(pbc3[:], lhsT=ones[:], rhs=b2_sb[:], start=True, stop=True)
            nc.vector.tensor_copy(b2b[:], pbc3[:])

            # ---- phase 1: raw/sraw + round-1 table (scaled by -nd[row]) ----
            for j in range(NB):
                xblk = xp.tile([128, FC * 128], F32, tag="x")
                nc.sync.dma_start(xblk[:].rearrange("k (c n) -> k c n", c=FC),
                                  xtb[j])
                pm = ps.tile([128, H], F32, tag="mm")
                for c in range(FC):
                    nc.tensor.matmul(pm[:], lhsT=xblk[:, c * 128:(c + 1) * 128],
                                     rhs=w1_sb[:, c * H:(c + 1) * H],
                                     start=(c == 0), stop=(c == FC - 1))
                rawa = sm.tile([128, H], F32, tag="rawa")
                nc.vector.tensor_add(rawa[:], pm[:], b1b[:])
                rawb = sm.tile([128, H], F32, tag="rawb")
                nc.scalar.activation(rawb[:], rawa[:],
                                     mybir.ActivationFunctionType.Relu)
                nc.vector.tensor_mul(sraw[:, j * H:(j + 1) * H], rawb[:], scale128[:])
                tbl = sm.tile([128, H], F32, tag="tbl")
                nc.vector.tensor_scalar_mul(tbl[:], rawb[:], ndn_sb[:, j:j + 1])
                nc.sync.dma_start(bounce1[j * 128:(j + 1) * 128, :], tbl[:])

            lagg = pp.tile([128, NB * H], F32)

            def subtile(pm, s, src, start, stop):
                gt = gp.tile([128, H], F32, tag="gt", name="gt")
                nc.gpsimd.indirect_dma_start(
                    out=gt[:], out_offset=None, in_=src[:, :],
                    in_offset=bass.IndirectOffsetOnAxis(
                        ap=idx_sb[:, s:s + 1], axis=0))
                sel = sp.tile([128, 128], F32, tag="sel", name="sel")
                nc.vector.tensor_tensor(
                    out=sel[:],
                    in0=lpos_sb[:, s:s + 1].to_broadcast([128, 128]),
                    in1=iota_f[:], op=mybir.AluOpType.is_equal)
                nc.tensor.matmul(pm[:], lhsT=sel[:], rhs=gt[:],
                                 start=start, stop=stop)

            # edges from this core's own rows: gather from the local bounce
            # buffer while the AllGather is still in flight
            def local_phase(bounce):
                nc.vector.memset(lagg[:], 0.0)
                s_off = 0
                for g in range(NG):
                    n_l = int(L_g[g])
                    if n_l > 0:
                        pm = ps.tile([128, H], F32, tag="mm", name="pm")
                        for t in range(n_l):
                            subtile(pm, s_off + t, bounce, t == 0, t == n_l - 1)
                        nc.vector.tensor_copy(lagg[:, g * H:(g + 1) * H], pm[:])
                    s_off += n_l + int(G_g[g])

            def global_phase(tfull, emit):
                s_off = 0
                for g in range(NG):
                    n_l, n_t = int(L_g[g]), int(G_g[g])
                    pm = ps.tile([128, H], F32, tag="mm", name="pm")
                    for t in range(n_t):
                        subtile(pm, s_off + n_l + t, tfull, t == 0, t == n_t - 1)
                    s_off += n_l + n_t
                    emit(g, pm)

            nc.gpsimd.collective_compute(
                "AllGather", mybir.AluOpType.bypass, replica_groups=groups,
                ins=[bounce1[:].opt()], outs=[t1full[:].opt()])
            local_phase(bounce1)

            def emit_r1(g, pm):
                h1b = sm.tile([128, H], F32, tag="h1b")
                nc.vector.tensor_add(h1b[:], pm[:], lagg[:, g * H:(g + 1) * H])
                nc.vector.tensor_scalar_mul(h1b[:], h1b[:], ndp_sb[:, g:g + 1])
                nc.vector.tensor_add(h1b[:], h1b[:], sraw[:, g * H:(g + 1) * H])
                tbl = sm.tile([128, H], F32, tag="tbl")
                nc.vector.tensor_scalar_mul(tbl[:], h1b[:], ndn_sb[:, g:g + 1])
                nc.sync.dma_start(bounce2[g * 128:(g + 1) * 128, :], tbl[:])

            global_phase(t1full, emit_r1)

            nc.gpsimd.collective_compute(
                "AllGather", mybir.AluOpType.bypass, replica_groups=groups,
                ins=[bounce2[:].opt()], outs=[t2full[:].opt()])
            local_phase(bounce2)

            def emit_r2(g, pm):
                h2b = sm.tile([128, H], F32, tag="h2b")
                nc.vector.tensor_add(h2b[:], pm[:], lagg[:, g * H:(g + 1) * H])
                nc.vector.tensor_scalar_mul(h2b[:], h2b[:], ndp_sb[:, g:g + 1])
                nc.vector.tensor_add(h2b[:], h2b[:], sraw[:, g * H:(g + 1) * H])
                nc.sync.dma_start(h2_o[g * 128:(g + 1) * 128, :], h2b[:])
                # logits = h2 @ W2.T + b2 ; log_softmax rowwise
                tp = psb.tile([H, 128], F32, tag="tp")
                nc.tensor.transpose(out=tp[:], in_=h2b[:], identity=ident[:])
                h2t = sm.tile([H, 128], F32, tag="h2t")
                nc.vector.tensor_copy(h2t[:], tp[:])
                lp = psb.tile([128, C], F32, tag="lg")
                nc.tensor.matmul(lp[:], lhsT=h2t[:], rhs=w2_sb[:], start=True,
                                 stop=True)
                lg = sm.tile([128, C], F32, tag="lg2")
                nc.vector.tensor_add(lg[:], lp[:], b2b[:])
                nmx = sm.tile([128, 1], F32, tag="nmx")
                nc.vector.reduce_max(nmx[:], lg[:], axis=mybir.AxisListType.X,
                                     negate=True)
                ex = sm.tile([128, C], F32, tag="ex")
                nc.scalar.activation(ex[:], lg[:],
                                     mybir.ActivationFunctionType.Exp,
                                     bias=nmx[:])
                ssum = sm.tile([128, 1], F32, tag="ssum")
                nc.vector.reduce_sum(ssum[:], ex[:], axis=mybir.AxisListType.X)
                lns = sm.tile([128, 1], F32, tag="lns")
                nc.scalar.activation(lns[:], ssum[:],
                                     mybir.ActivationFunctionType.Ln)
                ls = sm.tile([128, C], F32, tag="ls")
                nc.vector.tensor_scalar(
                    out=ls[:], in0=lg[:], scalar1=nmx[:], scalar2=lns[:],
                    op0=mybir.AluOpType.add, op1=mybir.AluOpType.subtract)
                nc.sync.dma_start(lsm_o[g * 128:(g + 1) * 128, :], ls[:])

            global_phase(t2full, emit_r2)

    nc.compile()
    return nc


def assemble(results, cfg, perms):
    lsm = np.concatenate(
        [r["lsm"][perms[k][:cfg.NPCR]] for k, r in enumerate(results)], axis=0)
    h2 = np.concatenate(
        [r["h2o"][perms[k][:cfg.NPCR]] for k, r in enumerate(results)], axis=0)
    return lsm, h2


def kernel(**inputs):
    cfg = REAL
    in_maps, S_g, perms = prep(inputs, cfg)
    nc = build(cfg, S_g)
    res = bass_utils.run_bass_kernel_spmd(
        nc, in_maps, core_ids=list(range(cfg.ncores)))
    return assemble(res.results, cfg, perms)
